# revision 5
# baseline (speedup 1.0000x reference)
"""AUGRU (attention-modulated GRU) Trainium2 Bass kernel.

Problem: B=4096, T=200, D=H=128.  For each t:
  z = sigmoid([x,h] @ Wz + bz); r = sigmoid([x,h] @ Wr + br)
  h~ = tanh([x, r*h] @ Wh + bh); zp = a_t * z; h' = (1-zp)*h + zp*h~

Sharding: data-parallel over batch, B/8 = 512 rows per NeuronCore.

Host-side prep (inside kernel(), before dispatch): x is transposed to
[T, D, B_shard] and cast to fp16 (the matmul moving operand; fp16's
10-bit mantissa keeps the 200-step state random-walk error ~4x below
bf16), attention scores to [T, B_shard] fp16, h0 to [H, B_shard] fp16,
weights split into x-part / h-part, fp16.

Per-core device layout: state hT [128(h), 512(b)] fp16 in SBUF, two
half-batch chains staggered half a step.  Gate matmuls keep the 128x128
weights stationary and stream xT_t / hT / (r*h)T into PSUM (fp32).
Biases ride the ACT engine's per-partition bias operand (bias indexes
h = partitions), so there are no bias matmuls.  The x-part matmuls for
step t+1 are emitted as PE filler between step t's latency-critical
h-part matmuls, keeping the in-order PE queue dense (better p-state,
no idle).  sigmoid r / z are separate ACTs (r first: r -> r*h -> h~
matmul -> tanh is the long pole).  a_t is replicated across partitions
by a chunked zero-stride DMA; q = a*z runs on GPSIMD; the tail
(d = h~-h, m = q*d, h' = h+m) on DVE in fp16.
"""

import numpy as np

B, T, D, H = 4096, 200, 128, 128
NCORES = 8
BS = B // NCORES            # 512 batch rows per core
C = 10                      # timestep chunk for attention staging

F16 = np.float16

_compiled = None


def _build(t_steps=T, chunk=C, nchains=2):
    import concourse.bass as bass
    import concourse.bacc as bacc
    import concourse.mybir as mybir
    from concourse.tile import TileContext
    from concourse.bass_types import AP

    fp32 = mybir.dt.float32
    fp16 = mybir.dt.float16
    Sigmoid = mybir.ActivationFunctionType.Sigmoid
    Tanh = mybir.ActivationFunctionType.Tanh

    assert t_steps % chunk == 0
    nchunks = t_steps // chunk

    nco = bacc.Bacc(
        "TRN2", target_bir_lowering=False, debug=False, num_devices=NCORES
    )
    xt_d = nco.dram_tensor("xt", [t_steps, D, BS], fp16, kind="ExternalInput")
    a_d = nco.dram_tensor("abf", [t_steps, BS], fp16, kind="ExternalInput")
    h0_d = nco.dram_tensor("h0t", [H, BS], fp16, kind="ExternalInput")
    wx_d = nco.dram_tensor("wx", [D, 3 * H], fp16, kind="ExternalInput")
    wh_d = nco.dram_tensor("wh", [H, 3 * H], fp16, kind="ExternalInput")
    b_d = nco.dram_tensor("bcol", [H, 4], fp32, kind="ExternalInput")
    out_d = nco.dram_tensor("out", [H, BS], fp16, kind="ExternalOutput")

    with TileContext(nco) as tc:
        with (
            tc.tile_pool(name="const", bufs=1) as constp,
            tc.tile_pool(name="xT", bufs=6) as xTp,
            tc.tile_pool(name="ab", bufs=2) as abp,
            tc.tile_pool(name="state", bufs=3) as statep,
            tc.tile_pool(name="tmp", bufs=3) as tmpp,
            tc.tile_pool(name="zr", bufs=2) as zrp,
            tc.tile_pool(name="ps_zr", bufs=2, space="PSUM") as ps_zr,
            tc.tile_pool(name="ps_p", bufs=2, space="PSUM") as ps_p,
        ):
            mm = nco.tensor.matmul

            # ---- constants ----
            wx_sb = constp.tile([128, 3 * H], fp16, tag="wx")
            nco.sync.dma_start(out=wx_sb[:], in_=wx_d.ap())
            wh_sb = constp.tile([128, 3 * H], fp16, tag="wh")
            nco.sync.dma_start(out=wh_sb[:], in_=wh_d.ap())
            b_sb = constp.tile([128, 4], fp32, tag="bcol")
            nco.sync.dma_start(out=b_sb[:], in_=b_d.ap())

            CW = BS // nchains  # chain width (batch columns per chain)
            hTs = []
            for c in range(nchains):
                hT = statep.tile([128, CW], fp16, tag=f"h{c}")
                nco.sync.dma_start(
                    out=hT[:], in_=h0_d.ap()[:, c * CW : (c + 1) * CW]
                )
                hTs.append(hT[:])

            # Chunked attention broadcast: one zero-stride DMA replicates
            # a[t0:t0+chunk, :] across all 128 partitions.
            def abload(t0):
                ab_ch = abp.tile([128, chunk, BS], fp16, tag="ab")
                asrc = a_d.ap()[t0 : t0 + chunk, :]
                asrc = AP(asrc.tensor, asrc.offset, [[0, 128]] + list(asrc.ap))
                nco.sync.dma_start(out=ab_ch[:], in_=asrc)
                return ab_ch

            ab_chunks = [None] * nchunks
            ab_chunks[0] = abload(0)
            if nchunks > 1:
                ab_chunks[1] = abload(chunk)

            st = [dict(hT=hTs[c], groups=[]) for c in range(nchains)]

            # p1x: x-part matmuls for (c, t) — open the PSUM groups.
            # Emitted one step AHEAD as filler between critical h-matmuls.
            def p1x(c, t, xT_t, which):
                s_ = st[c]
                cw = slice(c * CW, (c + 1) * CW)
                if which == "r":
                    zr_ps = ps_zr.tile([128, 2 * CW], fp32, tag=f"zrps{c}")
                    p_ps = ps_p.tile([128, CW], fp32, tag=f"pps{c}")
                    s_["groups"].append((zr_ps, p_ps))
                    mm(zr_ps[:, CW:], wx_sb[:, 128:256], xT_t[:, cw],
                       start=True, stop=False)
                elif which == "z":
                    zr_ps, _ = s_["groups"][-1]
                    mm(zr_ps[:, 0:CW], wx_sb[:, 0:128], xT_t[:, cw],
                       start=True, stop=False)
                else:
                    _, p_ps = s_["groups"][-1]
                    mm(p_ps[:], wx_sb[:, 256:384], xT_t[:, cw],
                       start=True, stop=False)

            # p1h: h-part matmuls close z|r; sigmoids (r first).
            def p1h(c, t):
                s_ = st[c]
                cw = slice(c * CW, (c + 1) * CW)
                s_["ab"] = ab_chunks[t // chunk][:, t % chunk, cw]
                zr_ps, p_ps = s_["groups"].pop(0)
                mm(zr_ps[:, CW:], wh_sb[:, 128:256], s_["hT"], start=False, stop=True)
                mm(zr_ps[:, 0:CW], wh_sb[:, 0:128], s_["hT"], start=False, stop=True)
                zr_bf = zrp.tile([128, 2 * CW], fp16, tag=f"zr{c}")
                nco.scalar.activation(
                    zr_bf[:, CW:], zr_ps[:, CW:], Sigmoid, bias=b_sb[:, 1:2]
                )
                nco.scalar.activation(
                    zr_bf[:, 0:CW], zr_ps[:, 0:CW], Sigmoid, bias=b_sb[:, 0:1]
                )
                s_["zr_ps"], s_["p_ps"], s_["zr_bf"] = zr_ps, p_ps, zr_bf

            def p2(c):
                s_ = st[c]
                rh = tmpp.tile([128, CW], fp16, tag=f"rh{c}")
                nco.vector.tensor_mul(rh[:], s_["zr_bf"][:, CW:], s_["hT"])
                # off the critical path: q = a * z on GPSIMD
                qq = tmpp.tile([128, CW], fp16, tag=f"q{c}")
                nco.gpsimd.tensor_mul(qq[:], s_["ab"], s_["zr_bf"][:, 0:CW])
                mm(s_["p_ps"][:], wh_sb[:, 256:384], rh[:], start=False, stop=True)
                ht_ = tmpp.tile([128, CW], fp16, tag=f"ht{c}")
                nco.scalar.activation(
                    ht_[:], s_["p_ps"][:], Tanh, bias=b_sb[:, 2:3]
                )
                s_["q"], s_["ht"] = qq, ht_

            def p3(c):
                s_ = st[c]
                dd = tmpp.tile([128, CW], fp16, tag=f"d{c}")
                nco.vector.tensor_sub(dd[:], s_["ht"][:], s_["hT"])
                mt = tmpp.tile([128, CW], fp16, tag=f"m{c}")
                nco.vector.tensor_mul(mt[:], s_["q"][:], dd[:])
                hT_new = statep.tile([128, CW], fp16, tag=f"h{c}")
                nco.vector.tensor_add(hT_new[:], s_["hT"], mt[:])
                hTs[c] = hT_new[:]
                s_["hT"] = hTs[c]

            def xload(t):
                # x_t [128(d), BS] — one DMA per step so consumers wait on
                # a single semaphore (large sprayed DMAs overflow the ISA
                # wait-slot budget of the consuming matmul)
                xT_t = xTp.tile([128, BS], fp16, tag="xT")
                nco.sync.dma_start(out=xT_t[:], in_=xt_d.ap()[t])
                return xT_t[:]

            if nchains == 1:
                xts = {0: xload(0)}
                for w in ("r", "z", "p"):
                    p1x(0, 0, xts[0], w)
                for t in range(t_steps):
                    nxt = t // chunk + 1
                    if t % chunk == 0 and nxt < nchunks and ab_chunks[nxt] is None:
                        ab_chunks[nxt] = abload(t + chunk)
                    if t + 1 < t_steps:
                        xts[t + 1] = xload(t + 1)
                    p1h(0, t)
                    if t + 1 < t_steps:
                        for w in ("r", "z", "p"):
                            p1x(0, t + 1, xts[t + 1], w)
                        del xts[t]
                    p2(0)
                    p3(0)
            else:
                # Software-pipelined half-step stagger: chain B runs half a
                # step behind A.  Step t+1's x-part matmuls are emitted as
                # PE filler between step t's critical h-part matmuls.
                xts = {0: xload(0), 1: xload(1) if t_steps > 1 else None}
                for c in range(nchains):
                    for w in ("r", "z", "p"):
                        p1x(c, 0, xts[0], w)
                for t in range(t_steps):
                    nxt = t // chunk + 1
                    if t % chunk == 0 and nxt < nchunks and ab_chunks[nxt] is None:
                        ab_chunks[nxt] = abload(t + chunk)
                    if t + 2 < t_steps:
                        xts[t + 2] = xload(t + 2)
                    p1h(0, t)
                    if t > 0:
                        p2(1)
                    if t + 1 < t_steps:
                        for w in ("r", "z", "p"):
                            p1x(0, t + 1, xts[t + 1], w)
                    p2(0)
                    if t > 0:
                        p3(1)
                    p1h(1, t)
                    if t + 1 < t_steps:
                        for w in ("r", "z", "p"):
                            p1x(1, t + 1, xts[t + 1], w)
                    p3(0)
                    xts.pop(t - 1, None)
                p2(1)
                p3(1)

            # ---- store final state transposed [H, BS] fp16; host flips ----
            for c in range(nchains):
                nco.gpsimd.dma_start(
                    out=out_d.ap()[:, c * CW : (c + 1) * CW], in_=hTs[c]
                )

    nco.compile()
    return nco


def _in_maps(inputs, t_steps=T):
    x = np.asarray(inputs["inputs"], np.float32)
    a = np.asarray(inputs["attention_scores"], np.float32)
    h0 = np.asarray(inputs["h0"], np.float32)
    Wz = np.asarray(inputs["Wz"], np.float32)
    Wr = np.asarray(inputs["Wr"], np.float32)
    Wh = np.asarray(inputs["Wh"], np.float32)
    wx = np.concatenate([Wz[:D], Wr[:D], Wh[:D]], axis=1).astype(F16)
    wh = np.concatenate([Wz[D:], Wr[D:], Wh[D:]], axis=1).astype(F16)
    bcol = np.zeros((H, 4), np.float32)
    for i, k in enumerate(("bz", "br", "bh")):
        bcol[:, i] = np.asarray(inputs[k], np.float32)
    maps = []
    for c in range(NCORES):
        sl = slice(c * BS, (c + 1) * BS)
        maps.append(
            {
                # [T, D, BS] fp16: host transpose + cast
                "xt": np.ascontiguousarray(
                    x[sl, :t_steps].transpose(1, 2, 0)
                ).astype(F16),
                "abf": np.ascontiguousarray(a[sl, :t_steps].T).astype(F16),
                "h0t": np.ascontiguousarray(h0[sl].T).astype(F16),
                "wx": wx,
                "wh": wh,
                "bcol": bcol,
            }
        )
    return maps


def kernel(**inputs):
    global _compiled
    from concourse.bass_utils import run_bass_kernel_spmd

    if _compiled is None:
        _compiled = _build()
    res = run_bass_kernel_spmd(_compiled, _in_maps(inputs), core_ids=list(range(NCORES)))
    return np.ascontiguousarray(
        np.concatenate(
            [np.asarray(r["out"]).astype(np.float32).T for r in res.results], axis=0
        )
    )


# revision 6
# speedup vs baseline: 1.0191x; 1.0191x over previous
"""AUGRU (attention-modulated GRU) Trainium2 Bass kernel.

Problem: B=4096, T=200, D=H=128.  For each t:
  z = sigmoid([x,h] @ Wz + bz); r = sigmoid([x,h] @ Wr + br)
  h~ = tanh([x, r*h] @ Wh + bh); zp = a_t * z; h' = (1-zp)*h + zp*h~

Sharding: data-parallel over batch, B/8 = 512 rows per NeuronCore.

Host-side prep (inside kernel(), before dispatch): x is transposed to
[T, D, B_shard] fp16 (the matmul moving operand; fp16's 10-bit mantissa
keeps the 200-step state rounding walk ~4x below bf16), attention
scores [T, B_shard] fp16, h0 [H, B_shard] fp16, weights split into
x-part / h-part fp16.

Per-core device layout: state hT [128(h), 512(b)] fp16 in SBUF, two
half-batch chains staggered half a step.  PSUM holds one bank per gate
per step ([128,512] fp32, double buffered = 6 banks): a single N=512
x-part matmul (start=True) opens each bank for BOTH chains, then each
chain's h-part matmul accumulates into its own half.  One accumulation
group per bank at a time — the PE's per-bank accumulation state does
not tolerate interleaved open groups in one bank (interleaving them
silently drops the earlier partial product).

Biases ride the ACT engine's per-partition bias operand (bias indexes
h = partitions), so there are no bias matmuls.  sigmoid r / z are
separate ACTs (r first: r -> r*h -> h~ matmul -> tanh is the long
pole).  a_t is replicated across partitions by a chunked zero-stride
DMA; q = a*z runs on GPSIMD; the tail (d = h~-h, m = q*d, h' = h+m)
on DVE in fp16.
"""

import numpy as np

B, T, D, H = 4096, 200, 128, 128
NCORES = 8
BS = B // NCORES            # 512 batch rows per core
C = 10                      # timestep chunk for attention staging

F16 = np.float16

_compiled = None


def _build(t_steps=T, chunk=C, nchains=2):
    import concourse.bass as bass
    import concourse.bacc as bacc
    import concourse.mybir as mybir
    from concourse.tile import TileContext
    from concourse.bass_types import AP

    fp32 = mybir.dt.float32
    fp16 = mybir.dt.float16
    Sigmoid = mybir.ActivationFunctionType.Sigmoid
    Tanh = mybir.ActivationFunctionType.Tanh

    assert t_steps % chunk == 0
    nchunks = t_steps // chunk

    nco = bacc.Bacc(
        "TRN2", target_bir_lowering=False, debug=False, num_devices=NCORES
    )
    xt_d = nco.dram_tensor("xt", [t_steps, D, BS], fp16, kind="ExternalInput")
    a_d = nco.dram_tensor("abf", [t_steps, BS], fp16, kind="ExternalInput")
    h0_d = nco.dram_tensor("h0t", [H, BS], fp16, kind="ExternalInput")
    wx_d = nco.dram_tensor("wx", [D, 3 * H], fp16, kind="ExternalInput")
    wh_d = nco.dram_tensor("wh", [H, 3 * H], fp16, kind="ExternalInput")
    b_d = nco.dram_tensor("bcol", [H, 4], fp32, kind="ExternalInput")
    out_d = nco.dram_tensor("out", [H, BS], fp16, kind="ExternalOutput")

    GSL = {"z": slice(0, 128), "r": slice(128, 256), "p": slice(256, 384)}

    with TileContext(nco) as tc:
        with (
            tc.tile_pool(name="const", bufs=1) as constp,
            tc.tile_pool(name="xT", bufs=6) as xTp,
            tc.tile_pool(name="ab", bufs=2) as abp,
            tc.tile_pool(name="state", bufs=3) as statep,
            tc.tile_pool(name="tmp", bufs=3) as tmpp,
            tc.tile_pool(name="zr", bufs=2) as zrp,
            tc.tile_pool(name="psg", bufs=2, space="PSUM") as psg,
        ):
            mm = nco.tensor.matmul

            # ---- constants ----
            wx_sb = constp.tile([128, 3 * H], fp16, tag="wx")
            nco.sync.dma_start(out=wx_sb[:], in_=wx_d.ap())
            wh_sb = constp.tile([128, 3 * H], fp16, tag="wh")
            nco.sync.dma_start(out=wh_sb[:], in_=wh_d.ap())
            b_sb = constp.tile([128, 4], fp32, tag="bcol")
            nco.sync.dma_start(out=b_sb[:], in_=b_d.ap())

            CW = BS // nchains  # chain width (batch columns per chain)
            hTs = []
            for c in range(nchains):
                hT = statep.tile([128, CW], fp16, tag=f"h{c}")
                nco.sync.dma_start(
                    out=hT[:], in_=h0_d.ap()[:, c * CW : (c + 1) * CW]
                )
                hTs.append(hT[:])

            # Chunked attention broadcast: one zero-stride DMA replicates
            # a[t0:t0+chunk, :] across all 128 partitions.
            def abload(t0):
                ab_ch = abp.tile([128, chunk, BS], fp16, tag="ab")
                asrc = a_d.ap()[t0 : t0 + chunk, :]
                asrc = AP(asrc.tensor, asrc.offset, [[0, 128]] + list(asrc.ap))
                nco.sync.dma_start(out=ab_ch[:], in_=asrc)
                return ab_ch

            ab_chunks = [None] * nchunks
            ab_chunks[0] = abload(0)
            if nchunks > 1:
                ab_chunks[1] = abload(chunk)

            # banks[t-parity] = {gate: psum tile [128, BS]}
            banks = {}
            st = [dict(hT=hTs[c]) for c in range(nchains)]

            # x-part matmul for gate g, step t: one N=512 matmul opens the
            # bank for both chains (single start per bank — the per-bank
            # accumulation state tolerates only one open group).
            def p1x(g, t, xT_t):
                bank = psg.tile([128, BS], fp32, tag=f"ps{g}")
                banks[(t, g)] = bank
                mm(bank[:], wx_sb[:, GSL[g]], xT_t, start=True, stop=False)

            # h-part matmuls close each chain's halves of z|r; sigmoids.
            def p1h(c, t):
                s_ = st[c]
                cw = slice(c * CW, (c + 1) * CW)
                s_["ab"] = ab_chunks[t // chunk][:, t % chunk, cw]
                zb, rb = banks[(t, "z")], banks[(t, "r")]
                mm(rb[:, cw], wh_sb[:, GSL["r"]], s_["hT"], start=False,
                   stop=(c == nchains - 1), skip_group_check=True)
                mm(zb[:, cw], wh_sb[:, GSL["z"]], s_["hT"], start=False,
                   stop=(c == nchains - 1), skip_group_check=True)
                zr_bf = zrp.tile([128, 2 * CW], fp16, tag=f"zr{c}")
                nco.scalar.activation(
                    zr_bf[:, CW:], rb[:, cw], Sigmoid, bias=b_sb[:, 1:2]
                )
                nco.scalar.activation(
                    zr_bf[:, 0:CW], zb[:, cw], Sigmoid, bias=b_sb[:, 0:1]
                )
                s_["zr_bf"] = zr_bf

            def p2(c, t):
                s_ = st[c]
                cw = slice(c * CW, (c + 1) * CW)
                rh = tmpp.tile([128, CW], fp16, tag=f"rh{c}")
                nco.vector.tensor_mul(rh[:], s_["zr_bf"][:, CW:], s_["hT"])
                # off the critical path: q = a * z on GPSIMD
                qq = tmpp.tile([128, CW], fp16, tag=f"q{c}")
                nco.gpsimd.tensor_mul(qq[:], s_["ab"], s_["zr_bf"][:, 0:CW])
                pb = banks[(t, "p")]
                mm(pb[:, cw], wh_sb[:, GSL["p"]], rh[:], start=False,
                   stop=(c == nchains - 1), skip_group_check=True)
                ht_ = tmpp.tile([128, CW], fp16, tag=f"ht{c}")
                nco.scalar.activation(
                    ht_[:], pb[:, cw], Tanh, bias=b_sb[:, 2:3]
                )
                s_["q"], s_["ht"] = qq, ht_

            def p3(c):
                s_ = st[c]
                dd = tmpp.tile([128, CW], fp16, tag=f"d{c}")
                nco.vector.tensor_sub(dd[:], s_["ht"][:], s_["hT"])
                mt = tmpp.tile([128, CW], fp16, tag=f"m{c}")
                nco.vector.tensor_mul(mt[:], s_["q"][:], dd[:])
                hT_new = statep.tile([128, CW], fp16, tag=f"h{c}")
                nco.vector.tensor_add(hT_new[:], s_["hT"], mt[:])
                hTs[c] = hT_new[:]
                s_["hT"] = hTs[c]

            def xload(t):
                xT_t = xTp.tile([128, BS], fp16, tag="xT")
                nco.sync.dma_start(out=xT_t[:], in_=xt_d.ap()[t])
                return xT_t[:]

            if nchains == 1:
                xts = {0: xload(0)}
                for g in ("r", "z", "p"):
                    p1x(g, 0, xts[0])
                for t in range(t_steps):
                    nxt = t // chunk + 1
                    if t % chunk == 0 and nxt < nchunks and ab_chunks[nxt] is None:
                        ab_chunks[nxt] = abload(t + chunk)
                    if t + 1 < t_steps:
                        xts[t + 1] = xload(t + 1)
                    p1h(0, t)
                    if t + 1 < t_steps:
                        for g in ("r", "z", "p"):
                            p1x(g, t + 1, xts[t + 1])
                    p2(0, t)
                    p3(0)
                    for g in ("z", "r", "p"):
                        banks.pop((t, g), None)
                    xts.pop(t, None)
            else:
                # Half-step stagger: chain B runs half a step behind A.
                # Step t+1's x-part matmuls are emitted as PE filler between
                # step t's critical h-part matmuls.
                xts = {0: xload(0)}
                if t_steps > 1:
                    xts[1] = xload(1)
                for g in ("r", "z", "p"):
                    p1x(g, 0, xts[0])
                for t in range(t_steps):
                    nxt = t // chunk + 1
                    if t % chunk == 0 and nxt < nchunks and ab_chunks[nxt] is None:
                        ab_chunks[nxt] = abload(t + chunk)
                    if t + 2 < t_steps:
                        xts[t + 2] = xload(t + 2)
                    p1h(0, t)
                    if t > 0:
                        p2(1, t - 1)
                    if t + 1 < t_steps:
                        p1x("r", t + 1, xts[t + 1])
                        p1x("z", t + 1, xts[t + 1])
                    p2(0, t)
                    if t > 0:
                        p3(1)
                    p1h(1, t)
                    if t + 1 < t_steps:
                        p1x("p", t + 1, xts[t + 1])
                    p3(0)
                    for g in ("z", "r", "p"):
                        banks.pop((t - 1, g), None)
                    xts.pop(t - 1, None)
                p2(1, t_steps - 1)
                p3(1)

            # ---- store final state transposed [H, BS] fp16; host flips ----
            for c in range(nchains):
                nco.gpsimd.dma_start(
                    out=out_d.ap()[:, c * CW : (c + 1) * CW], in_=hTs[c]
                )

    nco.compile()
    return nco


def _in_maps(inputs, t_steps=T):
    x = np.asarray(inputs["inputs"], np.float32)
    a = np.asarray(inputs["attention_scores"], np.float32)
    h0 = np.asarray(inputs["h0"], np.float32)
    Wz = np.asarray(inputs["Wz"], np.float32)
    Wr = np.asarray(inputs["Wr"], np.float32)
    Wh = np.asarray(inputs["Wh"], np.float32)
    wx = np.concatenate([Wz[:D], Wr[:D], Wh[:D]], axis=1).astype(F16)
    wh = np.concatenate([Wz[D:], Wr[D:], Wh[D:]], axis=1).astype(F16)
    bcol = np.zeros((H, 4), np.float32)
    for i, k in enumerate(("bz", "br", "bh")):
        bcol[:, i] = np.asarray(inputs[k], np.float32)
    maps = []
    for c in range(NCORES):
        sl = slice(c * BS, (c + 1) * BS)
        maps.append(
            {
                "xt": np.ascontiguousarray(
                    x[sl, :t_steps].transpose(1, 2, 0)
                ).astype(F16),
                "abf": np.ascontiguousarray(a[sl, :t_steps].T).astype(F16),
                "h0t": np.ascontiguousarray(h0[sl].T).astype(F16),
                "wx": wx,
                "wh": wh,
                "bcol": bcol,
            }
        )
    return maps


def kernel(**inputs):
    global _compiled
    from concourse.bass_utils import run_bass_kernel_spmd

    if _compiled is None:
        _compiled = _build()
    res = run_bass_kernel_spmd(_compiled, _in_maps(inputs), core_ids=list(range(NCORES)))
    return np.ascontiguousarray(
        np.concatenate(
            [np.asarray(r["out"]).astype(np.float32).T for r in res.results], axis=0
        )
    )


# revision 10
# speedup vs baseline: 1.0742x; 1.0541x over previous
"""AUGRU (attention-modulated GRU) Trainium2 Bass kernel.

Problem: B=4096, T=200, D=H=128.  For each t:
  z = sigmoid([x,h] @ Wz + bz); r = sigmoid([x,h] @ Wr + br)
  h~ = tanh([x, r*h] @ Wh + bh); zp = a_t * z; h' = (1-zp)*h + zp*h~

Sharding: data-parallel over batch, B/8 = 512 rows per NeuronCore.

Host-side prep (inside kernel(), before dispatch): x is transposed to
[T, D, B_shard] fp16 (the matmul moving operand; fp16's 10-bit mantissa
keeps the 200-step state rounding walk ~4x below bf16), attention
scores [T, B_shard] fp16, h0 [H, B_shard] fp16, weights split into
x-part / h-part fp16.

Per-core device layout: state hT [128(h), 512(b)] fp16 in SBUF, two
half-batch chains staggered half a step.  PSUM holds one bank per gate
per step ([128,512] fp32, double buffered = 6 banks): a single N=512
x-part matmul (start=True) opens each bank for BOTH chains, then each
chain's h-part matmul accumulates into its own half.  One accumulation
group per bank at a time — the PE's per-bank accumulation state does
not tolerate interleaved open groups in one bank (interleaving them
silently drops the earlier partial product).

Biases ride the ACT engine's per-partition bias operand (bias indexes
h = partitions), so there are no bias matmuls.  sigmoid r / z are
separate ACTs (r first: r -> r*h -> h~ matmul -> tanh is the long
pole).  a_t is replicated across partitions by a chunked zero-stride
DMA; q = a*z runs on GPSIMD; the tail (d = h~-h, m = q*d, h' = h+m)
on DVE in fp16.
"""

import numpy as np

B, T, D, H = 4096, 200, 128, 128
NCORES = 8
BS = B // NCORES            # 512 batch rows per core
C = 10                      # timestep chunk for attention staging

F16 = np.float16

_compiled = None


def _build(t_steps=T, chunk=C, nchains=2, ndum=12):
    import concourse.bass as bass
    import concourse.bacc as bacc
    import concourse.mybir as mybir
    from concourse.tile import TileContext
    from concourse.bass_types import AP

    fp32 = mybir.dt.float32
    fp16 = mybir.dt.float16
    Sigmoid = mybir.ActivationFunctionType.Sigmoid
    Tanh = mybir.ActivationFunctionType.Tanh

    assert t_steps % chunk == 0
    nchunks = t_steps // chunk

    nco = bacc.Bacc(
        "TRN2", target_bir_lowering=False, debug=False, num_devices=NCORES
    )
    xt_d = nco.dram_tensor("xt", [t_steps, D, BS], fp16, kind="ExternalInput")
    a_d = nco.dram_tensor("abf", [t_steps, BS], fp16, kind="ExternalInput")
    h0_d = nco.dram_tensor("h0t", [H, BS], fp16, kind="ExternalInput")
    wx_d = nco.dram_tensor("wx", [D, 3 * H], fp16, kind="ExternalInput")
    wh_d = nco.dram_tensor("wh", [H, 3 * H], fp16, kind="ExternalInput")
    b_d = nco.dram_tensor("bcol", [H, 4], fp32, kind="ExternalInput")
    out_d = nco.dram_tensor("out", [H, BS], fp16, kind="ExternalOutput")

    GSL = {"z": slice(0, 128), "r": slice(128, 256), "p": slice(256, 384)}

    with TileContext(nco) as tc:
        with (
            tc.tile_pool(name="const", bufs=1) as constp,
            tc.tile_pool(name="xT", bufs=6) as xTp,
            tc.tile_pool(name="ab", bufs=2) as abp,
            tc.tile_pool(name="state", bufs=3) as statep,
            tc.tile_pool(name="tmp", bufs=3) as tmpp,
            tc.tile_pool(name="zr", bufs=2) as zrp,
            tc.tile_pool(name="psg", bufs=2, space="PSUM") as psg,
            tc.tile_pool(name="psd", bufs=2, space="PSUM") as psd,
        ):
            mm = nco.tensor.matmul

            # ---- constants ----
            wx_sb = constp.tile([128, 3 * H], fp16, tag="wx")
            nco.sync.dma_start(out=wx_sb[:], in_=wx_d.ap())
            wh_sb = constp.tile([128, 3 * H], fp16, tag="wh")
            nco.sync.dma_start(out=wh_sb[:], in_=wh_d.ap())
            b_sb = constp.tile([128, 4], fp32, tag="bcol")
            nco.sync.dma_start(out=b_sb[:], in_=b_d.ap())

            CW = BS // nchains  # chain width (batch columns per chain)
            hTs = []
            for c in range(nchains):
                hT = statep.tile([128, CW], fp16, tag=f"h{c}")
                nco.sync.dma_start(
                    out=hT[:], in_=h0_d.ap()[:, c * CW : (c + 1) * CW]
                )
                hTs.append(hT[:])

            # Chunked attention broadcast: one zero-stride DMA replicates
            # a[t0:t0+chunk, :] across all 128 partitions.
            def abload(t0):
                ab_ch = abp.tile([128, chunk, BS], fp16, tag="ab")
                asrc = a_d.ap()[t0 : t0 + chunk, :]
                asrc = AP(asrc.tensor, asrc.offset, [[0, 128]] + list(asrc.ap))
                nco.sync.dma_start(out=ab_ch[:], in_=asrc)
                return ab_ch

            ab_chunks = [None] * nchunks
            ab_chunks[0] = abload(0)
            if nchunks > 1:
                ab_chunks[1] = abload(chunk)

            # banks[t-parity] = {gate: psum tile [128, BS]}
            banks = {}
            st = [dict(hT=hTs[c]) for c in range(nchains)]

            # x-part matmul for gate g, step t: one N=512 matmul opens the
            # bank for both chains (single start per bank — the per-bank
            # accumulation state tolerates only one open group).
            def p1x(g, t, xT_t):
                bank = psg.tile([128, BS], fp32, tag=f"ps{g}")
                banks[(t, g)] = bank
                mm(bank[:], wx_sb[:, GSL[g]], xT_t, start=True, stop=False)

            # h-part matmuls close each chain's halves of z|r; sigmoids.
            def p1h(c, t):
                s_ = st[c]
                cw = slice(c * CW, (c + 1) * CW)
                s_["ab"] = ab_chunks[t // chunk][:, t % chunk, cw]
                zb, rb = banks[(t, "z")], banks[(t, "r")]
                mm(rb[:, cw], wh_sb[:, GSL["r"]], s_["hT"], start=False,
                   stop=(c == nchains - 1), skip_group_check=True)
                mm(zb[:, cw], wh_sb[:, GSL["z"]], s_["hT"], start=False,
                   stop=(c == nchains - 1), skip_group_check=True)
                zr_bf = zrp.tile([128, 2 * CW], fp16, tag=f"zr{c}")
                nco.scalar.activation(
                    zr_bf[:, CW:], rb[:, cw], Sigmoid, bias=b_sb[:, 1:2]
                )
                nco.scalar.activation(
                    zr_bf[:, 0:CW], zb[:, cw], Sigmoid, bias=b_sb[:, 0:1]
                )
                s_["zr_bf"] = zr_bf

            def p2(c, t):
                s_ = st[c]
                cw = slice(c * CW, (c + 1) * CW)
                rh = tmpp.tile([128, CW], fp16, tag=f"rh{c}")
                nco.vector.tensor_mul(rh[:], s_["zr_bf"][:, CW:], s_["hT"])
                # off the critical path: q = a * z on GPSIMD
                qq = tmpp.tile([128, CW], fp16, tag=f"q{c}")
                nco.gpsimd.tensor_mul(qq[:], s_["ab"], s_["zr_bf"][:, 0:CW])
                pb = banks[(t, "p")]
                mm(pb[:, cw], wh_sb[:, GSL["p"]], rh[:], start=False,
                   stop=(c == nchains - 1), skip_group_check=True)
                ht_ = tmpp.tile([128, CW], fp16, tag=f"ht{c}")
                nco.scalar.activation(
                    ht_[:], pb[:, cw], Tanh, bias=b_sb[:, 2:3]
                )
                s_["q"], s_["ht"] = qq, ht_

            def p3(c):
                s_ = st[c]
                dd = tmpp.tile([128, CW], fp16, tag=f"d{c}")
                nco.vector.tensor_sub(dd[:], s_["ht"][:], s_["hT"])
                mt = tmpp.tile([128, CW], fp16, tag=f"m{c}")
                nco.vector.tensor_mul(mt[:], s_["q"][:], dd[:])
                hT_new = statep.tile([128, CW], fp16, tag=f"h{c}")
                nco.vector.tensor_add(hT_new[:], s_["hT"], mt[:])
                hTs[c] = hT_new[:]
                s_["hT"] = hTs[c]

            def xload(t):
                xT_t = xTp.tile([128, BS], fp16, tag="xT")
                nco.sync.dma_start(out=xT_t[:], in_=xt_d.ap()[t])
                return xT_t[:]

            # Keep-warm filler: the PE drops from 2.4GHz to 1.2GHz whenever
            # its pipeline drains (>2x on every real matmul).  Harmless
            # always-ready matmuls into spare PSUM banks keep it hot; the
            # scheduler slots them into gaps where the PE would idle.
            def dummies(n):
                for _ in range(n):
                    db = psd.tile([128, 384], fp32, tag="dum")
                    mm(db[:], wx_sb[:, 0:128], wh_sb[:], start=True, stop=True)

            if nchains == 1:
                xts = {0: xload(0)}
                for g in ("r", "z", "p"):
                    p1x(g, 0, xts[0])
                for t in range(t_steps):
                    nxt = t // chunk + 1
                    if t % chunk == 0 and nxt < nchunks and ab_chunks[nxt] is None:
                        ab_chunks[nxt] = abload(t + chunk)
                    if t + 1 < t_steps:
                        xts[t + 1] = xload(t + 1)
                    p1h(0, t)
                    if t + 1 < t_steps:
                        for g in ("r", "z", "p"):
                            p1x(g, t + 1, xts[t + 1])
                    p2(0, t)
                    p3(0)
                    for g in ("z", "r", "p"):
                        banks.pop((t, g), None)
                    xts.pop(t, None)
            else:
                # Half-step stagger: chain B runs half a step behind A.
                # Step t+1's x-part matmuls are emitted as PE filler between
                # step t's critical h-part matmuls.
                xts = {0: xload(0)}
                if t_steps > 1:
                    xts[1] = xload(1)
                for g in ("r", "z", "p"):
                    p1x(g, 0, xts[0])
                for t in range(t_steps):
                    nxt = t // chunk + 1
                    if t % chunk == 0 and nxt < nchunks and ab_chunks[nxt] is None:
                        ab_chunks[nxt] = abload(t + chunk)
                    if t + 2 < t_steps:
                        xts[t + 2] = xload(t + 2)
                    n1 = ndum // 4
                    p1h(0, t)
                    dummies(n1)
                    if t > 0:
                        p2(1, t - 1)
                    if t + 1 < t_steps:
                        p1x("r", t + 1, xts[t + 1])
                        p1x("z", t + 1, xts[t + 1])
                    dummies(n1)
                    p2(0, t)
                    if t > 0:
                        p3(1)
                    dummies(n1)
                    p1h(1, t)
                    if t + 1 < t_steps:
                        p1x("p", t + 1, xts[t + 1])
                    dummies(ndum - 3 * n1)
                    p3(0)
                    for g in ("z", "r", "p"):
                        banks.pop((t - 1, g), None)
                    xts.pop(t - 1, None)
                p2(1, t_steps - 1)
                p3(1)

            # ---- store final state transposed [H, BS] fp16; host flips ----
            for c in range(nchains):
                nco.gpsimd.dma_start(
                    out=out_d.ap()[:, c * CW : (c + 1) * CW], in_=hTs[c]
                )

    nco.compile()
    return nco


def _in_maps(inputs, t_steps=T):
    x = np.asarray(inputs["inputs"], np.float32)
    a = np.asarray(inputs["attention_scores"], np.float32)
    h0 = np.asarray(inputs["h0"], np.float32)
    Wz = np.asarray(inputs["Wz"], np.float32)
    Wr = np.asarray(inputs["Wr"], np.float32)
    Wh = np.asarray(inputs["Wh"], np.float32)
    wx = np.concatenate([Wz[:D], Wr[:D], Wh[:D]], axis=1).astype(F16)
    wh = np.concatenate([Wz[D:], Wr[D:], Wh[D:]], axis=1).astype(F16)
    bcol = np.zeros((H, 4), np.float32)
    for i, k in enumerate(("bz", "br", "bh")):
        bcol[:, i] = np.asarray(inputs[k], np.float32)
    maps = []
    for c in range(NCORES):
        sl = slice(c * BS, (c + 1) * BS)
        maps.append(
            {
                "xt": np.ascontiguousarray(
                    x[sl, :t_steps].transpose(1, 2, 0)
                ).astype(F16),
                "abf": np.ascontiguousarray(a[sl, :t_steps].T).astype(F16),
                "h0t": np.ascontiguousarray(h0[sl].T).astype(F16),
                "wx": wx,
                "wh": wh,
                "bcol": bcol,
            }
        )
    return maps


def kernel(**inputs):
    global _compiled
    from concourse.bass_utils import run_bass_kernel_spmd

    if _compiled is None:
        _compiled = _build()
    res = run_bass_kernel_spmd(_compiled, _in_maps(inputs), core_ids=list(range(NCORES)))
    return np.ascontiguousarray(
        np.concatenate(
            [np.asarray(r["out"]).astype(np.float32).T for r in res.results], axis=0
        )
    )
